# revision 16
# baseline (speedup 1.0000x reference)
"""Trainium2 Bass kernel v5 for CommittorNetBP (pairwise min-image env sum + tiny MLP).

Mathematically equivalent reformulation:

 1. wrap(dx)^2 ~= B0 + sum_n Bn cos(2*pi*n*dx/L): pairwise d2 is an inner
    product of trig embeddings, shipped fp16 with a SYMMETRIC split: both
    factors carry sqrt(|Bn|), the sign rides on the stationary side, and
    hi/lo compensation rows for n=1,2 keep fp16 rounding out (K=120 rows).
    The fp16 product matrix is exactly symmetric, so row sums == col sums.
 2. i-blocks are tiled 3-per-[128, 1536] PSUM tile (linear in block index
    beta = 4*b + k, crossing batch boundaries), double-buffered in 6 PSUM
    banks, leaving one bank for csum.  One exp ACTIVATE per tile: the ACT
    engine is the bottleneck and runs just 22 instructions.
 3. Envelope v = (u + C)*u via DVE tensor_scalar (4x mode) + tensor_tensor
    (2x mode).  inputt = column sums of v: one-hot stationary matmuls
    accumulate all 64 blocks into a single csum [16, 512] PSUM group
    (PE has idle streaming capacity; DVE/ACT reductions would be 1x).
    Colsum matmuls trail the pair matmuls by 2 tiles so PE never blocks
    on the DVE chain.
 4. MLP tail transposed: csum -> ACT copy -> 4 PE transposes -> bf16 casts
    -> hT[n, b] accumulates bf16 hi/lo bias + 8 w1t-chunk matmuls; relu;
    z[1, 16] via two [128, 1] fp32 w2 matmuls; tanh sigmoid; y is [1, 16]
    on one partition = single-descriptor output DMA.
 5. Only the exp_and_others ACT table is used (exp/relu/tanh/copy): one
    table load, preloaded during the DMA wait; PE warmed up with dummies.
 6. DMA: E chunks on the sync queue, Ew + weights on the gpsimd queue,
    batch-0-first then growing chunks; scalar/vector queues stay clean.

Sharding: pure data parallel, batch 128 -> 8 cores x 16.
"""

import numpy as np
import ml_dtypes

# ---------------------------------------------------------------- constants
L = 10.0
PI = float(np.pi)
NP = 512
BTOT = 128
NCORES = 8
BLOC = BTOT // NCORES   # 16
NH = 16
K0 = 6 * NH             # 96 base embedding rows
KC = 24                 # compensation rows (n=1,2: 12 lo-stationary + 12 lo-moving)
K = K0 + KC             # 120
NUM_NODES = 256
XW = BLOC * NP          # 8192
NBLK = 4 * BLOC         # 64 i-blocks, beta = 4*b + k
# Tile pattern: alternate 4-block [128, 2048] and 3-block [128, 1536] PSUM
# tiles (8KB + 6KB + 2KB csum = 16KB PSUM exactly); 9 pairs cover 63 blocks,
# one final 1-block tile covers the last.  18+1 exp ACTIVATEs total.
TILE_SIZES = [4, 3] * 9 + [1]
TILE_STARTS = [sum(TILE_SIZES[:i]) for i in range(len(TILE_SIZES))]
NTILE = len(TILE_SIZES)  # 19

B_HARM = [
    8.336507198660753, -10.134305777836879, 2.5283072633082164,
    -1.1207547738471013, 0.6351791173907125, -0.41237594667899846,
    0.28478810229590223, -0.20163605059415754, 0.15059719920404221,
    -0.12490354747428888, 0.11118898587488348, -0.09477489833163562,
    0.06985971056432684, -0.041620415059490684, 0.018837434788739185,
    -0.005869820105041354, 0.0009762178400180537,
]

# envelope fit f(t) ~= CW0 + CW1*e^{-A_ENV t} + CW2*e^{-2 A_ENV t}
A_ENV = 0.70
CW0 = -1.18809612e-06
CW1 = -3.03076726e-02
CW2 = 1.03030886e+00
C_STT = float(np.float32(CW1 / CW2))   # v = (u + C_STT) * u
EXP_BIAS = -A_ENV * 3.0 * B_HARM[0]

N_WARM = 8

f32 = np.float32
bf16 = ml_dtypes.bfloat16


def _host_embeddings(x):
    """x: [BLOC, NP, 3] fp32 -> E (moving) and Ew (stationary) [120, 8192] fp16.

    SYMMETRIC scheme: both factors carry sqrt(|Bn|), the harmonic's sign
    rides on the stationary side only, so t[i,j] == t[j,i] exactly.
    Rows 0..95: hi parts for all n; rows 96..119: hi/lo compensation pairs
    for n=1,2 => sum = EH*EH + EL*EH + EH*EL (exact to ~2^-22 for n=1,2).
    """
    xs = np.transpose(np.asarray(x, np.float64), (2, 0, 1)).reshape(3, XW)
    E = np.zeros((K, XW), np.float16)
    Ew = np.zeros((K, XW), np.float16)
    lo = 0
    for c in range(3):
        for n in range(1, NH + 1):
            s = np.sqrt(abs(B_HARM[n]))
            sgn = np.float16(np.sign(B_HARM[n]))
            for p, phase in enumerate((0.25, 0.0)):
                row = c * 32 + (n - 1) * 2 + p
                th = 2.0 * np.pi * (n * xs[c] / L + phase)
                e = s * np.sin(th)
                eh = np.float16(e)
                E[row] = eh
                Ew[row] = sgn * eh
                if n <= 2:
                    el = np.float16(e - np.float64(eh))
                    E[96 + lo] = eh           # moving hi pairs stationary lo
                    Ew[96 + lo] = sgn * el
                    E[108 + lo] = el          # moving lo pairs stationary hi
                    Ew[108 + lo] = sgn * eh
                    lo += 1
    assert lo == 12
    return E, Ew


_CACHE = {}


def _build_program():
    import concourse.bacc as bacc
    import concourse.mybir as mybir
    import concourse.tile as tile

    nc = bacc.Bacc("TRN2", target_bir_lowering=False, debug=False,
                   num_devices=NCORES)
    dt = mybir.dt
    AF = mybir.ActivationFunctionType
    ALU = mybir.AluOpType

    E_d = nc.declare_dram_parameter("E", (K, XW), dt.float16, isOutput=False)
    Ew_d = nc.declare_dram_parameter("Ew", (K, XW), dt.float16, isOutput=False)
    w1t_d = nc.declare_dram_parameter("w1t", (NP, NUM_NODES), dt.bfloat16, isOutput=False)
    b1pb_d = nc.declare_dram_parameter("b1pb", (1, NUM_NODES), dt.bfloat16, isOutput=False)
    b1pl_d = nc.declare_dram_parameter("b1pl", (1, NUM_NODES), dt.bfloat16, isOutput=False)
    w2t_d = nc.declare_dram_parameter("w2t", (128, 2), dt.float32, isOutput=False)
    eye_d = nc.declare_dram_parameter("eye16", (16, 16), dt.float32, isOutput=False)
    y_d = nc.declare_dram_parameter("y", (1, BLOC), dt.float32, isOutput=True)

    with tile.TileContext(nc) as tc:
        with tc.tile_pool(name="const", bufs=1) as cpool:
            # ---------------- input DMAs ----------------
            # Early chunks spread across FOUR queues so no single queue's
            # config+transfer serialization starves the first tiles:
            #   sync:   E[0:512], E[512:1024], E[4096:8192]
            #   gpsimd: Ew[0:512], Ew[512:1024], Ew[4096:8192], weights, y
            #   scalar: E[1024:2048], Ew[1024:2048]  (before the exp stream)
            #   vector: E[2048:4096], Ew[2048:4096]  (before the TS/TT stream)
            E_s = cpool.tile([K, XW], dt.float16)
            Ew_s = cpool.tile([K, XW], dt.float16)
            nc.sync.dma_start(E_s[:, 0:512], E_d[:, 0:512])
            nc.gpsimd.dma_start(Ew_s[:, 0:512], Ew_d[:, 0:512])
            nc.sync.dma_start(E_s[:, 512:1024], E_d[:, 512:1024])
            nc.gpsimd.dma_start(Ew_s[:, 512:1024], Ew_d[:, 512:1024])
            nc.scalar.dma_start(E_s[:, 1024:2048], E_d[:, 1024:2048])
            nc.scalar.dma_start(Ew_s[:, 1024:2048], Ew_d[:, 1024:2048])
            nc.sync.dma_start(E_s[:, 2048:4096], E_d[:, 2048:4096])
            nc.gpsimd.dma_start(Ew_s[:, 2048:4096], Ew_d[:, 2048:4096])
            nc.sync.dma_start(E_s[:, 4096:8192], E_d[:, 4096:8192])
            nc.gpsimd.dma_start(Ew_s[:, 4096:8192], Ew_d[:, 4096:8192])
            w1t_s = cpool.tile([128, 4 * NUM_NODES], dt.bfloat16)
            for c in range(4):
                nc.gpsimd.dma_start(
                    w1t_s[:, c * NUM_NODES:(c + 1) * NUM_NODES],
                    w1t_d[c * 128:(c + 1) * 128, :])
            b1pb = cpool.tile([1, NUM_NODES], dt.bfloat16)
            nc.gpsimd.dma_start(b1pb[:], b1pb_d[:])
            b1pl = cpool.tile([1, NUM_NODES], dt.bfloat16)
            nc.gpsimd.dma_start(b1pl[:], b1pl_d[:])
            w2t_s = cpool.tile([128, 2], dt.float32)
            nc.gpsimd.dma_start(w2t_s[:], w2t_d[:])
            eye_s = cpool.tile([16, 16], dt.float32)
            nc.gpsimd.dma_start(eye_s[:], eye_d[:])

            warm = cpool.tile([1, 128], dt.bfloat16)
            nc.vector.memset(warm[:], 1.0)
            ebias = cpool.tile([128, 1], dt.float32)
            nc.vector.memset(ebias[:], EXP_BIAS)
            ones16 = cpool.tile([1, BLOC], dt.bfloat16)
            nc.vector.memset(ones16[:], 1.0)
            oh_bf = cpool.tile([128, 31], dt.bfloat16)
            nc.vector.memset(oh_bf[:], 0.0)
            nc.vector.memset(oh_bf[:, 15:16], 1.0)
            wexp_i = cpool.tile([1, 16], dt.float32)
            nc.vector.memset(wexp_i[:], 0.0)
            wexp_o = cpool.tile([1, 16], dt.float32)
            # preload the exp ACT table during the DMA wait (the only table)
            nc.scalar.activation(wexp_o[:], wexp_i[:], AF.Exp)

            # ---------------- PE warmup during DMA wait ----------------
            with tc.tile_pool(name="wp", bufs=1, space="PSUM") as wp:
                wt = wp.tile([1, 128], dt.float32)
                for _ in range(N_WARM):
                    nc.tensor.matmul(wt[:], warm[0:1, 0:1], warm[0:1, :],
                                     start=True, stop=True,
                                     skip_group_check=True)

            # ---- main loop: alternating 4/3-block PSUM tiles, 2 pools ----
            with tc.tile_pool(name="cs", bufs=1, space="PSUM") as cspool:
              csum = cspool.tile([BLOC, NP], dt.float32)
              with (
                tc.tile_pool(name="tpA", bufs=1, space="PSUM") as tpA,
                tc.tile_pool(name="tpB", bufs=1, space="PSUM") as tpB,
                tc.tile_pool(name="uu", bufs=3) as upool,
                tc.tile_pool(name="ss", bufs=2) as spool,
                tc.tile_pool(name="vv", bufs=4) as vpool,
              ):
                v_l = [None] * NTILE

                def tile_blocks(m):
                    return list(range(TILE_STARTS[m],
                                      TILE_STARTS[m] + TILE_SIZES[m]))

                def emit_tile(m):
                    betas = tile_blocks(m)
                    w = 512 * len(betas)
                    if m % 2 == 0:
                        t = tpA.tile([128, 2048], dt.float32, tag="t")
                    else:
                        t = tpB.tile([128, 1536], dt.float32, tag="t")
                    for j, beta in enumerate(betas):
                        b, k = divmod(beta, 4)
                        nc.tensor.matmul(
                            t[:, j * 512:(j + 1) * 512],
                            Ew_s[:, b * 512 + k * 128: b * 512 + k * 128 + 128],
                            E_s[:, b * 512:(b + 1) * 512],
                            start=True, stop=True)
                    u = upool.tile([128, 2048], dt.bfloat16, tag="u")
                    nc.scalar.activation(u[:, 0:w], t[:, 0:w], AF.Exp,
                                         scale=-A_ENV, bias=ebias[:, 0:1])
                    s = spool.tile([128, 2048], dt.bfloat16, tag="s")
                    nc.vector.tensor_scalar(s[:, 0:w], u[:, 0:w], C_STT, None,
                                            ALU.add)
                    v = vpool.tile([128, 2048], dt.bfloat16, tag="v")
                    nc.vector.tensor_tensor(v[:, 0:w], s[:, 0:w], u[:, 0:w],
                                            ALU.mult)
                    v_l[m] = v

                def emit_colsum(m):
                    v = v_l[m]
                    for j, beta in enumerate(tile_blocks(m)):
                        b, k = divmod(beta, 4)
                        nc.tensor.matmul(
                            csum[:, :], oh_bf[:, 15 - b:31 - b],
                            v[:, j * 512:(j + 1) * 512],
                            start=(beta == 0), stop=(beta == NBLK - 1),
                            skip_group_check=True)
                    v_l[m] = None

                for m in range(NTILE):
                    emit_tile(m)
                    if m >= 2:
                        emit_colsum(m - 2)
                emit_colsum(NTILE - 2)
                emit_colsum(NTILE - 1)

              # ---------------- MLP tail (transposed) ----------------
              with (
                tc.tile_pool(name="trp", bufs=4, space="PSUM") as trpool,
                tc.tile_pool(name="hp", bufs=1, space="PSUM") as hpool,
                tc.tile_pool(name="tail", bufs=1) as tail,
              ):
                scopy = tail.tile([BLOC, NP], dt.float32)
                nc.scalar.activation(scopy[:], csum[:], AF.Copy)
                it_l = []
                for c in range(4):
                    tp = trpool.tile([128, BLOC], dt.float32, tag="tp")
                    nc.tensor.transpose(
                        tp[:], scopy[:, c * 128:(c + 1) * 128], eye_s[:])
                    itc = tail.tile([128, BLOC], dt.bfloat16, name=f"it{c}")
                    nc.vector.tensor_copy(itc[:], tp[:])
                    it_l.append(itc)
                hT = hpool.tile([128, 2 * BLOC], dt.float32)
                for half in range(2):
                    hsl = slice(half * BLOC, (half + 1) * BLOC)
                    nc.tensor.matmul(hT[:, hsl],
                                     b1pb[0:1, half * 128:(half + 1) * 128],
                                     ones16[0:1, :],
                                     start=True, stop=False,
                                     skip_group_check=True)
                    nc.tensor.matmul(hT[:, hsl],
                                     b1pl[0:1, half * 128:(half + 1) * 128],
                                     ones16[0:1, :],
                                     start=False, stop=False,
                                     skip_group_check=True)
                    for c in range(4):
                        nc.tensor.matmul(
                            hT[:, hsl],
                            w1t_s[:, c * 256 + half * 128:
                                  c * 256 + half * 128 + 128],
                            it_l[c][:],
                            start=False, stop=(c == 3),
                            skip_group_check=True)
                hrT = tail.tile([128, 2 * BLOC], dt.float32)
                nc.scalar.activation(hrT[:], hT[:], AF.Relu)
                z = hpool.tile([1, BLOC], dt.float32)
                for half in range(2):
                    nc.tensor.matmul(z[0:1, :], w2t_s[:, half:half + 1],
                                     hrT[:, half * BLOC:(half + 1) * BLOC],
                                     start=(half == 0), stop=(half == 1),
                                     skip_group_check=True)
                th = tail.tile([1, BLOC], dt.float32)
                nc.scalar.activation(th[:], z[:], AF.Tanh, scale=0.5)
                ys = tail.tile([1, BLOC], dt.float32)
                nc.vector.tensor_scalar(ys[:], th[:], 0.5, 0.5,
                                        ALU.mult, ALU.add)
                nc.gpsimd.dma_start(y_d[:], ys[:])

    nc.finalize()
    return nc


def _get_program():
    if "nc" not in _CACHE:
        _CACHE["nc"] = _build_program()
    return _CACHE["nc"]


def _make_in_maps(x, W1, b1, W2):
    W1 = np.asarray(W1, f32)
    w1t = np.ascontiguousarray(W1.T * f32(CW2)).astype(bf16)
    b1p = (np.asarray(b1, f32)
           + (NP * f32(CW0) - 1.0) * W1.sum(axis=1)).reshape(1, NUM_NODES).astype(f32)
    b1pb = b1p.astype(bf16)
    b1pl = (b1p - b1pb.astype(f32)).astype(bf16)
    w2t = np.ascontiguousarray(
        np.asarray(W2, f32).reshape(2, 128).T).astype(f32)
    eye16 = np.eye(16, dtype=f32)
    x = np.asarray(x, f32)
    in_maps = []
    for c in range(NCORES):
        E, Ew = _host_embeddings(x[c * BLOC:(c + 1) * BLOC])
        in_maps.append({"E": E, "Ew": Ew, "w1t": w1t, "b1pb": b1pb,
                        "b1pl": b1pl, "w2t": w2t, "eye16": eye16})
    return in_maps


def kernel(x, W1, b1, W2, _trace=False, _trace_kwargs=None):
    from concourse.bass_utils import run_bass_kernel_spmd

    nc = _get_program()
    in_maps = _make_in_maps(x, W1, b1, W2)
    res = run_bass_kernel_spmd(nc, in_maps, list(range(NCORES)),
                               trace=_trace, **(_trace_kwargs or {}))
    out = np.concatenate([res.results[c]["y"].reshape(BLOC, 1)
                          for c in range(NCORES)], axis=0)
    if _trace:
        _CACHE["last_result"] = res
    return out.astype(f32)
